# revision 26
# baseline (speedup 1.0000x reference)
"""Trainium2 Bass kernel for nn_LA_283467842715.

Math (per batch b, head h of 16, each head owning 128 contiguous channels):
  means/maxs over (128 group channels x 2x2 patch) -> [B,16,4,4]
  tiny MLP (16->1 conv, relu, 1->16 conv) on means and maxs, fused by a
  2->1 conv, bilinear-upsampled 4x4->8x8, sigmoid -> gate
  out = x * (1 + gate[b, h, y, x])

Layout: SBUF tile [128, 8192] per chunk of 8 batches; partition p = b*16+h,
free = c*64 + y*8 + x (each partition row is one contiguous 32KB HBM block).
Sharding: pure data parallel, 32 batches per core across 8 cores.

Engine split per chunk (free-dim reduces are DVE-only on TRN2; the shared
DVE/GpSimd SBUF port means GpSimd work and DVE 2-input work must together
fit the chunk budget, so the reduces run 1-port on DVE and the multiply
runs on GpSimd):
  DVE    : step1 patch-reduces (sum+max) per c-slice, writing the
           intermediate TRANSPOSED [p,i,j,c] so step2's c-reduction reads
           contiguously; step2 c-reduces -> sm [128, 32]. DVE is the
           critical path: ~22.2us/chunk, scheduled gap-free.
  GpSimd : (gate+1)*x tensor_mul in 4 c-slices (each slice's DMA-out
           overlaps the next slice's multiply); Stat expand.
           The LAST chunk's multiply runs on the then-idle DVE as
           scalar_tensor_tensor (2x faster/elem) to shorten the tail.
  PE     : the whole MLP as two tiny matmuls
           mm1: out[k,b] = sum_p sm[p,k] * W1mov[p,b],  W1mov = onehot(b)*w1[h]
           mm2: out[(b,h),s] = sum_k Stat[k,(b,h)] * K2e2[k,s]
           (upsample matrix + wv fused into K2e2, w2 fused into Stat)
  ACT    : relu (with 1/512 mean scale as per-partition scale), sigmoid,
           gate+1 copy; also issues all output DMAs (scalar HWDGE queue)
           so they never head-of-line-block input DMAs on the sync queue.
"""

import sys

if "/opt/trn_rl_repo" not in sys.path:
    sys.path.insert(0, "/opt/trn_rl_repo")

import numpy as np

HEAD = 16
B, C, H, W = 256, 2048, 8, 8
NCORES = 8
BPC = B // NCORES          # 32 batches per core
CHUNK_B = 8                # batches per SBUF chunk (8*16 heads = 128 partitions)
NCHUNK = BPC // CHUNK_B    # 4
C16 = C // HEAD            # 128 channels per head group
SPAT = H * W               # 64
FREE = C16 * SPAT          # 8192 f32 per partition
NCONST = 91

LAST_EXEC_NS = None        # filled when trace=True


def _upsample_matrix():
    """U[8,4]: bilinear 4->8, half-pixel centers (align_corners=False)."""
    U = np.zeros((8, 4), dtype=np.float64)
    for y in range(8):
        src = (y + 0.5) / 2.0 - 0.5
        i0 = int(np.floor(src))
        t = src - i0
        U[y, min(max(i0, 0), 3)] += 1.0 - t
        U[y, min(max(i0 + 1, 0), 3)] += t
    return U


def _pack_params(w1, b1, w2, b2, wv, bv):
    w1 = np.asarray(w1, np.float64).reshape(HEAD)
    w2 = np.asarray(w2, np.float64).reshape(HEAD)
    b2 = np.asarray(b2, np.float64).reshape(HEAD)
    wv = np.asarray(wv, np.float64).reshape(2)
    bv = float(np.asarray(bv, np.float64))
    b1 = float(np.asarray(b1, np.float64))

    U = _upsample_matrix()
    # K2[g, s] = U[y,i] * U[x,j], g = i*4+j, s = y*8+x
    K2 = np.einsum("yi,xj->ijyx", U, U).reshape(16, 64)

    p = np.arange(128)
    k = np.arange(32)
    # One packed const tensor [128, 91] -> one small DMA.
    #   cols 0:8          W1mov[p, b'] = (p//16 == b') * w1[p%16]
    #   cols 8:24  (0:32) w2blk[k, h]  = w2[h]
    #   col  24    (0:32) scalecol     = 1/512 for k<16 (mean path) else 1
    #   col  25    (0:32) b1col        = b1
    #   col  26           betacol[p]   = (wv0+wv1)*b2[p%16] + bv
    #   cols 27:91 (0:32) K2e2[k, s]   = (wv0 if k<16 else wv1) * K2[k%16, s]
    CONST = np.zeros((128, NCONST))
    CONST[:, 0:8] = (p[:, None] // 16 == np.arange(8)[None, :]) * w1[p % 16][:, None]
    CONST[0:32, 8:24] = w2[None, :]
    CONST[0:32, 24] = np.where(k < 16, 1.0 / 512.0, 1.0)
    CONST[0:32, 25] = b1
    CONST[:, 26] = (wv[0] + wv[1]) * b2[p % 16] + bv
    CONST[0:32, 27:91] = np.where(k[:, None] < 16, wv[0], wv[1]) * K2[k % 16, :]
    return {"consts": np.ascontiguousarray(CONST, np.float32)}


def _split_multi_waits(nc, mybir):
    """Walrus codegen on this path only supports one sync-wait per
    instruction. Run the official Bacc lowering passes (the same ones
    Bacc.compile() uses) to hoist extra waits onto InstEventSemaphore."""
    import bass_rust as _bass_rust

    _bass_rust.move_matmul_waits_to_ldweights(nc.m)
    _bass_rust.generate_event_semaphores(nc)


def _build():
    import concourse.bass as bass
    import concourse.tile as tile
    from concourse import mybir

    f32 = mybir.dt.float32
    nc = bass.Bass()

    xd = nc.dram_tensor("x", [NCHUNK, 128, FREE], f32, kind="ExternalInput")
    od = nc.dram_tensor("out", [NCHUNK, 128, FREE], f32, kind="ExternalOutput")
    call = nc.dram_tensor("consts", [128, NCONST], f32, kind="ExternalInput")

    AF = mybir.ActivationFunctionType
    ALU = mybir.AluOpType
    AX = mybir.AxisListType

    with tile.TileContext(nc) as tc:
        with (
            tc.tile_pool(name="singles", bufs=1) as singles,
            tc.tile_pool(name="xin", bufs=4) as xpool,
            tc.tile_pool(name="mid", bufs=2) as mid,
            tc.tile_pool(name="small", bufs=2) as small,
            tc.tile_pool(name="psum", bufs=2, space="PSUM") as psum,
        ):
            # Consts go on the scalar HWDGE queue; the sync (SP) queue is
            # reserved for input DMAs so a stalled output DMA can never
            # head-of-line-block the next chunk's input.
            s_all = singles.tile([128, NCONST], f32)
            nc.scalar.dma_start(out=s_all, in_=call[:, :])
            s_W1mov = s_all[:, 0:8]
            s_w2blk = s_all[0:32, 8:24]
            s_scalecol = s_all[0:32, 24:25]
            s_b1col = s_all[0:32, 25:26]
            s_betacol = s_all[:, 26:27]
            s_K2e2 = s_all[0:32, 27:91]

            for ci in range(NCHUNK):
                # DMA the chunk in c-slices so step 1 starts after the first
                # slice has landed. The first chunk uses finer slices to pull
                # the start of the DVE chain (the critical path) earlier.
                X = xpool.tile([128, FREE], f32, tag="X")
                NQ = 8 if ci == 0 else 4
                QF = FREE // NQ
                QC = C16 // NQ
                for qf in range(NQ):
                    nc.sync.dma_start(out=X[:, qf * QF:(qf + 1) * QF],
                                      in_=xd[ci, :, qf * QF:(qf + 1) * QF])

                # Step 1 (per c-slice): reduce the 2x2 patch (dy,dx).
                # free idx = c*64 + i*16 + dy*8 + j*2 + dx; (c,i) merges to
                # one stride-16 dim. Output is stored TRANSPOSED as
                # [p, i, j, c] so step 2's c-reduction reads contiguously.
                # Chunk 0 matches the fine DMA slicing (early start); later
                # chunks use halves to amortize per-op dispatch overhead
                # (each half's dependency covers two landed DMA quarters).
                t2s = mid.tile([128, 4, 4, C16], f32, tag="t2s")
                t2m = mid.tile([128, 4, 4, C16], f32, tag="t2m")
                sm = small.tile([128, 32], f32, tag="sm")
                s16 = sm[:, 0:16].rearrange("p (i j) -> p i j", i=4)
                m16 = sm[:, 16:32].rearrange("p (i j) -> p i j", i=4)
                if ci <= 1:
                    # Chunks 0-1: GpSimd is idle until the first gates exist,
                    # so it pre-folds the dx pair of the SUM tree (elementwise
                    # add — the only TT alu-op walrus accepts on Pool besides
                    # mult — slice by slice as the DMA lands) while the DVE
                    # runs the max tree. The DVE then finishes the sum tree
                    # with one cheap fully-contiguous dy-reduce. Takes ~5us
                    # per chunk off the DVE chain (the critical path).
                    NSM = NQ if ci == 0 else 2
                    SMF = FREE // NSM
                    SMC = C16 // NSM
                    for qf in range(NSM):
                        c0 = qf * SMC
                        Xh = X[:, qf * SMF:(qf + 1) * SMF].rearrange(
                            "p (ci dy j dx) -> p ci j dy dx",
                            ci=SMC * 4, dy=2, j=4, dx=2
                        )
                        t2m_o = t2m[:, :, :, c0:c0 + SMC].rearrange(
                            "p i j c -> p c i j")
                        nc.vector.reduce_max(out=t2m_o, in_=Xh, axis=AX.XY)
                    fsum = mid.tile([128, 512, 4, 2], f32, tag="fsum")
                    for qf in range(NQ):
                        Xq = X[:, qf * QF:(qf + 1) * QF].rearrange(
                            "p (ci dy j dx) -> p ci j dy dx",
                            ci=QC * 4, dy=2, j=4, dx=2
                        )
                        ci0 = qf * QC * 4
                        fq = fsum[:, ci0:ci0 + QC * 4, :, :]
                        nc.gpsimd.tensor_add(
                            fq,
                            Xq[:, :, :, :, 0:1].squeeze(4),
                            Xq[:, :, :, :, 1:2].squeeze(4),
                        )
                    nc.vector.reduce_max(out=m16, in_=t2m[:, :, :, :],
                                         axis=AX.X)
                    # dy-reduce of the folded tree: storage [p, ci, j, dy]
                    # read as [p, c, i, j, dy] -> fully sequential.
                    f_r = fsum[:, :, :, :].rearrange(
                        "p (c i) j dy -> p c i j dy", c=C16)
                    t2s_all = t2s[:, :, :, :].rearrange("p i j c -> p c i j")
                    nc.vector.reduce_sum(out=t2s_all, in_=f_r, axis=AX.X)
                    nc.vector.reduce_sum(out=s16, in_=t2s[:, :, :, :],
                                         axis=AX.X)
                else:
                    NS1 = 2
                    SF = FREE // NS1
                    SC = C16 // NS1
                    for qf in range(NS1):
                        c0 = qf * SC
                        Xh = X[:, qf * SF:(qf + 1) * SF].rearrange(
                            "p (ci dy j dx) -> p ci j dy dx",
                            ci=SC * 4, dy=2, j=4, dx=2
                        )
                        t2s_o = t2s[:, :, :, c0:c0 + SC].rearrange(
                            "p i j c -> p c i j")
                        t2m_o = t2m[:, :, :, c0:c0 + SC].rearrange(
                            "p i j c -> p c i j")
                        nc.vector.reduce_sum(out=t2s_o, in_=Xh, axis=AX.XY)
                        nc.vector.reduce_max(out=t2m_o, in_=Xh, axis=AX.XY)
                    # Step 2: reduce c (innermost, contiguous) -> sm [128,32]
                    # (cols 0:16 sums, 16:32 maxes).
                    nc.vector.reduce_sum(out=s16, in_=t2s[:, :, :, :],
                                         axis=AX.X)
                    nc.vector.reduce_max(out=m16, in_=t2m[:, :, :, :],
                                         axis=AX.X)

                # mm1: hpreT[k, b] = sum_p sm[p, k] * W1mov[p, b]
                hpreT = psum.tile([32, 8], f32, tag="hpreT")
                nc.tensor.matmul(
                    out=hpreT[:, :], lhsT=sm[:, :], rhs=s_W1mov,
                    start=True, stop=True,
                )

                # hcatT = relu(scale*hpreT + b1); scale folds the /512 of the
                # mean path (rows 0:16).
                hcatT = small.tile([32, 8], f32, tag="hcatT")
                nc.scalar.activation(
                    hcatT[:, :], hpreT[:, :], AF.Relu,
                    bias=s_b1col, scale=s_scalecol,
                )

                # Stat[k, (b,h)] = hcatT[k, b] * w2[h]  (on GpSimd: keeps the
                # DVE reduce chain, which is the critical path, uninterrupted.
                # Last chunk: the chain is over, DVE is idle -> fewer hops.)
                stat = small.tile([32, 128], f32, tag="stat")
                stat3 = stat[:, :].rearrange("p (b h) -> p b h", h=16)
                h_bc = hcatT[:, :].unsqueeze(2).broadcast_to([32, 8, 16])
                w2_bc = s_w2blk.unsqueeze(1).broadcast_to([32, 8, 16])
                if ci == NCHUNK - 1:
                    nc.vector.tensor_mul(stat3, h_bc, w2_bc)
                else:
                    nc.gpsimd.tensor_mul(stat3, h_bc, w2_bc)

                # mm2: pF[(b,h), s] = sum_k Stat[k, (b,h)] * K2e2[k, s]
                pF = psum.tile([128, 64], f32, tag="pF")
                nc.tensor.matmul(
                    out=pF[:, :], lhsT=stat[:, :], rhs=s_K2e2,
                    start=True, stop=True,
                )

                # gate = sigmoid(pF + beta), then +1 on the idle ACT engine
                # (the last chunk folds the +1 into its DVE STT instead)
                gate = small.tile([128, 64], f32, tag="gate")
                nc.scalar.activation(gate[:, :], pF[:, :], AF.Sigmoid,
                                     bias=s_betacol)
                gate1 = None
                if ci != NCHUNK - 1:
                    gate1 = small.tile([128, 64], f32, tag="gate1")
                    nc.scalar.activation(gate1[:, :], gate[:, :], AF.Copy,
                                         bias=1.0)

                # out = (gate + 1) * x, gate broadcast over the 128 group
                # chans. Chunks 0..2 run it on GpSimd (the DVE is busy with
                # the next chunk's reduces, and 1-input DVE reduces leave the
                # shared DVE/GpSimd SBUF port free). The LAST chunk runs it
                # on the now-idle DVE via scalar_tensor_tensor, which is 2x
                # faster per element and shortens the tail. Sliced in 4
                # along c so each slice's DMA-out overlaps the next slice's
                # multiply.
                last = (ci == NCHUNK - 1)
                NSL = 8 if last else 4
                SL = FREE // NSL
                CSL = C16 // NSL
                for q in range(NSL):
                    f0 = q * SL
                    X3q = X[:, f0:f0 + SL].rearrange("p (c s) -> p c s", c=CSL)
                    if last:
                        g_bcq0 = gate[:, :].unsqueeze(1).broadcast_to(
                            [128, CSL, SPAT])
                        nc.vector.scalar_tensor_tensor(
                            out=X3q, in0=g_bcq0, scalar=1.0, in1=X3q,
                            op0=ALU.add, op1=ALU.mult,
                        )
                    else:
                        g_bcq = gate1[:, :].unsqueeze(1).broadcast_to(
                            [128, CSL, SPAT])
                        nc.gpsimd.tensor_mul(X3q, g_bcq, X3q)
                    nc.scalar.dma_start(out=od[ci, :, f0:f0 + SL],
                                        in_=X[:, f0:f0 + SL])

    _split_multi_waits(nc, mybir)
    return nc


def kernel(x, w1, b1, w2, b2, wv, bv, trace=False):
    global LAST_EXEC_NS
    from concourse.bass_utils import run_bass_kernel_spmd

    x = np.ascontiguousarray(np.asarray(x, np.float32))
    consts = _pack_params(w1, b1, w2, b2, wv, bv)

    nc = _build()

    in_maps = []
    for i in range(NCORES):
        shard = x[i * BPC:(i + 1) * BPC]  # [32, 2048, 8, 8]
        m = {"x": np.ascontiguousarray(shard.reshape(NCHUNK, 128, FREE))}
        m.update(consts)
        in_maps.append(m)

    res = run_bass_kernel_spmd(nc, in_maps, core_ids=list(range(NCORES)),
                               trace=trace)
    LAST_EXEC_NS = res.exec_time_ns

    out = np.empty((B, C, H, W), np.float32)
    for i, r in enumerate(res.results):
        out[i * BPC:(i + 1) * BPC] = r["out"].reshape(BPC, C, H, W)
    return out


# revision 27
# speedup vs baseline: 1.0322x; 1.0322x over previous
"""Trainium2 Bass kernel for nn_LA_283467842715.

Math (per batch b, head h of 16, each head owning 128 contiguous channels):
  means/maxs over (128 group channels x 2x2 patch) -> [B,16,4,4]
  tiny MLP (16->1 conv, relu, 1->16 conv) on means and maxs, fused by a
  2->1 conv, bilinear-upsampled 4x4->8x8, sigmoid -> gate
  out = x * (1 + gate[b, h, y, x])

Layout: SBUF tile [128, 8192] per chunk of 8 batches; partition p = b*16+h,
free = c*64 + y*8 + x (each partition row is one contiguous 32KB HBM block).
Sharding: pure data parallel, 32 batches per core across 8 cores.

Engine split per chunk (free-dim reduces are DVE-only on TRN2; the shared
DVE/GpSimd SBUF port means GpSimd work and DVE 2-input work must together
fit the chunk budget, so the reduces run 1-port on DVE and the multiply
runs on GpSimd):
  DVE    : step1 patch-reduces (sum+max) per c-slice, writing the
           intermediate TRANSPOSED [p,i,j,c] so step2's c-reduction reads
           contiguously; step2 c-reduces -> sm [128, 32]. DVE is the
           critical path: ~22.2us/chunk, scheduled gap-free.
  GpSimd : (gate+1)*x tensor_mul in 4 c-slices (each slice's DMA-out
           overlaps the next slice's multiply); Stat expand.
           The LAST chunk's multiply runs on the then-idle DVE as
           scalar_tensor_tensor (2x faster/elem) to shorten the tail.
  PE     : the whole MLP as two tiny matmuls
           mm1: out[k,b] = sum_p sm[p,k] * W1mov[p,b],  W1mov = onehot(b)*w1[h]
           mm2: out[(b,h),s] = sum_k Stat[k,(b,h)] * K2e2[k,s]
           (upsample matrix + wv fused into K2e2, w2 fused into Stat)
  ACT    : relu (with 1/512 mean scale as per-partition scale), sigmoid,
           gate+1 copy; also issues all output DMAs (scalar HWDGE queue)
           so they never head-of-line-block input DMAs on the sync queue.
"""

import sys

if "/opt/trn_rl_repo" not in sys.path:
    sys.path.insert(0, "/opt/trn_rl_repo")

import numpy as np

HEAD = 16
B, C, H, W = 256, 2048, 8, 8
NCORES = 8
BPC = B // NCORES          # 32 batches per core
CHUNK_B = 8                # batches per SBUF chunk (8*16 heads = 128 partitions)
NCHUNK = BPC // CHUNK_B    # 4
C16 = C // HEAD            # 128 channels per head group
SPAT = H * W               # 64
FREE = C16 * SPAT          # 8192 f32 per partition
NCONST = 91

LAST_EXEC_NS = None        # filled when trace=True


def _upsample_matrix():
    """U[8,4]: bilinear 4->8, half-pixel centers (align_corners=False)."""
    U = np.zeros((8, 4), dtype=np.float64)
    for y in range(8):
        src = (y + 0.5) / 2.0 - 0.5
        i0 = int(np.floor(src))
        t = src - i0
        U[y, min(max(i0, 0), 3)] += 1.0 - t
        U[y, min(max(i0 + 1, 0), 3)] += t
    return U


def _pack_params(w1, b1, w2, b2, wv, bv):
    w1 = np.asarray(w1, np.float64).reshape(HEAD)
    w2 = np.asarray(w2, np.float64).reshape(HEAD)
    b2 = np.asarray(b2, np.float64).reshape(HEAD)
    wv = np.asarray(wv, np.float64).reshape(2)
    bv = float(np.asarray(bv, np.float64))
    b1 = float(np.asarray(b1, np.float64))

    U = _upsample_matrix()
    # K2[g, s] = U[y,i] * U[x,j], g = i*4+j, s = y*8+x
    K2 = np.einsum("yi,xj->ijyx", U, U).reshape(16, 64)

    p = np.arange(128)
    k = np.arange(32)
    # One packed const tensor [128, 91] -> one small DMA.
    #   cols 0:8          W1mov[p, b'] = (p//16 == b') * w1[p%16]
    #   cols 8:24  (0:32) w2blk[k, h]  = w2[h]
    #   col  24    (0:32) scalecol     = 1/512 for k<16 (mean path) else 1
    #   col  25    (0:32) b1col        = b1
    #   col  26           betacol[p]   = (wv0+wv1)*b2[p%16] + bv
    #   cols 27:91 (0:32) K2e2[k, s]   = (wv0 if k<16 else wv1) * K2[k%16, s]
    CONST = np.zeros((128, NCONST))
    CONST[:, 0:8] = (p[:, None] // 16 == np.arange(8)[None, :]) * w1[p % 16][:, None]
    CONST[0:32, 8:24] = w2[None, :]
    CONST[0:32, 24] = np.where(k < 16, 1.0 / 512.0, 1.0)
    CONST[0:32, 25] = b1
    CONST[:, 26] = (wv[0] + wv[1]) * b2[p % 16] + bv
    CONST[0:32, 27:91] = np.where(k[:, None] < 16, wv[0], wv[1]) * K2[k % 16, :]
    return {"consts": np.ascontiguousarray(CONST, np.float32)}


def _split_multi_waits(nc, mybir):
    """Walrus codegen on this path only supports one sync-wait per
    instruction. Run the official Bacc lowering passes (the same ones
    Bacc.compile() uses) to hoist extra waits onto InstEventSemaphore."""
    import bass_rust as _bass_rust

    _bass_rust.move_matmul_waits_to_ldweights(nc.m)
    _bass_rust.generate_event_semaphores(nc)


def _build():
    import concourse.bass as bass
    import concourse.tile as tile
    from concourse import mybir

    f32 = mybir.dt.float32
    nc = bass.Bass()

    xd = nc.dram_tensor("x", [NCHUNK, 128, FREE], f32, kind="ExternalInput")
    od = nc.dram_tensor("out", [NCHUNK, 128, FREE], f32, kind="ExternalOutput")
    call = nc.dram_tensor("consts", [128, NCONST], f32, kind="ExternalInput")

    AF = mybir.ActivationFunctionType
    ALU = mybir.AluOpType
    AX = mybir.AxisListType

    with tile.TileContext(nc) as tc:
        with (
            tc.tile_pool(name="singles", bufs=1) as singles,
            tc.tile_pool(name="xin", bufs=4) as xpool,
            tc.tile_pool(name="mid", bufs=2) as mid,
            tc.tile_pool(name="small", bufs=2) as small,
            tc.tile_pool(name="psum", bufs=2, space="PSUM") as psum,
        ):
            # Consts go on the scalar HWDGE queue; the sync (SP) queue is
            # reserved for input DMAs so a stalled output DMA can never
            # head-of-line-block the next chunk's input.
            s_all = singles.tile([128, NCONST], f32)
            nc.scalar.dma_start(out=s_all, in_=call[:, :])
            s_W1mov = s_all[:, 0:8]
            s_w2blk = s_all[0:32, 8:24]
            s_scalecol = s_all[0:32, 24:25]
            s_b1col = s_all[0:32, 25:26]
            s_betacol = s_all[:, 26:27]
            s_K2e2 = s_all[0:32, 27:91]

            for ci in range(NCHUNK):
                # DMA the chunk in c-slices so step 1 starts after the first
                # slice has landed. The first chunk uses finer slices to pull
                # the start of the DVE chain (the critical path) earlier.
                X = xpool.tile([128, FREE], f32, tag="X")
                NQ = 8 if ci == 0 else 4
                QF = FREE // NQ
                QC = C16 // NQ
                for qf in range(NQ):
                    nc.sync.dma_start(out=X[:, qf * QF:(qf + 1) * QF],
                                      in_=xd[ci, :, qf * QF:(qf + 1) * QF])

                # Step 1 (per c-slice): reduce the 2x2 patch (dy,dx).
                # free idx = c*64 + i*16 + dy*8 + j*2 + dx; (c,i) merges to
                # one stride-16 dim. Output is stored TRANSPOSED as
                # [p, i, j, c] so step 2's c-reduction reads contiguously.
                # Chunk 0 matches the fine DMA slicing (early start); later
                # chunks use halves to amortize per-op dispatch overhead
                # (each half's dependency covers two landed DMA quarters).
                t2s = mid.tile([128, 4, 4, C16], f32, tag="t2s")
                t2m = mid.tile([128, 4, 4, C16], f32, tag="t2m")
                sm = small.tile([128, 32], f32, tag="sm")
                s16 = sm[:, 0:16].rearrange("p (i j) -> p i j", i=4)
                m16 = sm[:, 16:32].rearrange("p (i j) -> p i j", i=4)
                if ci == 0:
                    # Chunk 0: GpSimd is idle until the first gate exists, so
                    # it pre-folds the dx pair of the SUM tree (elementwise
                    # add — the only TT alu-op walrus accepts on Pool besides
                    # mult — slice by slice as the DMA lands) while the DVE
                    # runs the max tree. The DVE then finishes the sum tree
                    # with one cheap fully-contiguous dy-reduce. (Extending
                    # this to chunk 1 was tried and measured neutral: GpSimd
                    # program order places its folds after chunk 0's
                    # multiplies, too late to relieve the DVE.)
                    NSM = NQ
                    SMF = FREE // NSM
                    SMC = C16 // NSM
                    for qf in range(NSM):
                        c0 = qf * SMC
                        Xh = X[:, qf * SMF:(qf + 1) * SMF].rearrange(
                            "p (ci dy j dx) -> p ci j dy dx",
                            ci=SMC * 4, dy=2, j=4, dx=2
                        )
                        t2m_o = t2m[:, :, :, c0:c0 + SMC].rearrange(
                            "p i j c -> p c i j")
                        nc.vector.reduce_max(out=t2m_o, in_=Xh, axis=AX.XY)
                    fsum = mid.tile([128, 512, 4, 2], f32, tag="fsum")
                    for qf in range(NQ):
                        Xq = X[:, qf * QF:(qf + 1) * QF].rearrange(
                            "p (ci dy j dx) -> p ci j dy dx",
                            ci=QC * 4, dy=2, j=4, dx=2
                        )
                        ci0 = qf * QC * 4
                        fq = fsum[:, ci0:ci0 + QC * 4, :, :]
                        nc.gpsimd.tensor_add(
                            fq,
                            Xq[:, :, :, :, 0:1].squeeze(4),
                            Xq[:, :, :, :, 1:2].squeeze(4),
                        )
                    nc.vector.reduce_max(out=m16, in_=t2m[:, :, :, :],
                                         axis=AX.X)
                    # dy-reduce of the folded tree: storage [p, ci, j, dy]
                    # read as [p, c, i, j, dy] -> fully sequential.
                    f_r = fsum[:, :, :, :].rearrange(
                        "p (c i) j dy -> p c i j dy", c=C16)
                    t2s_all = t2s[:, :, :, :].rearrange("p i j c -> p c i j")
                    nc.vector.reduce_sum(out=t2s_all, in_=f_r, axis=AX.X)
                    nc.vector.reduce_sum(out=s16, in_=t2s[:, :, :, :],
                                         axis=AX.X)
                else:
                    NS1 = 2
                    SF = FREE // NS1
                    SC = C16 // NS1
                    for qf in range(NS1):
                        c0 = qf * SC
                        Xh = X[:, qf * SF:(qf + 1) * SF].rearrange(
                            "p (ci dy j dx) -> p ci j dy dx",
                            ci=SC * 4, dy=2, j=4, dx=2
                        )
                        t2s_o = t2s[:, :, :, c0:c0 + SC].rearrange(
                            "p i j c -> p c i j")
                        t2m_o = t2m[:, :, :, c0:c0 + SC].rearrange(
                            "p i j c -> p c i j")
                        nc.vector.reduce_sum(out=t2s_o, in_=Xh, axis=AX.XY)
                        nc.vector.reduce_max(out=t2m_o, in_=Xh, axis=AX.XY)
                    # Step 2: reduce c (innermost, contiguous) -> sm [128,32]
                    # (cols 0:16 sums, 16:32 maxes).
                    nc.vector.reduce_sum(out=s16, in_=t2s[:, :, :, :],
                                         axis=AX.X)
                    nc.vector.reduce_max(out=m16, in_=t2m[:, :, :, :],
                                         axis=AX.X)

                # mm1: hpreT[k, b] = sum_p sm[p, k] * W1mov[p, b]
                hpreT = psum.tile([32, 8], f32, tag="hpreT")
                nc.tensor.matmul(
                    out=hpreT[:, :], lhsT=sm[:, :], rhs=s_W1mov,
                    start=True, stop=True,
                )

                # hcatT = relu(scale*hpreT + b1); scale folds the /512 of the
                # mean path (rows 0:16).
                hcatT = small.tile([32, 8], f32, tag="hcatT")
                nc.scalar.activation(
                    hcatT[:, :], hpreT[:, :], AF.Relu,
                    bias=s_b1col, scale=s_scalecol,
                )

                # Stat[k, (b,h)] = hcatT[k, b] * w2[h]  (on GpSimd: keeps the
                # DVE reduce chain, which is the critical path, uninterrupted.
                # Last chunk: the chain is over, DVE is idle -> fewer hops.)
                stat = small.tile([32, 128], f32, tag="stat")
                stat3 = stat[:, :].rearrange("p (b h) -> p b h", h=16)
                h_bc = hcatT[:, :].unsqueeze(2).broadcast_to([32, 8, 16])
                w2_bc = s_w2blk.unsqueeze(1).broadcast_to([32, 8, 16])
                if ci == NCHUNK - 1:
                    nc.vector.tensor_mul(stat3, h_bc, w2_bc)
                else:
                    nc.gpsimd.tensor_mul(stat3, h_bc, w2_bc)

                # mm2: pF[(b,h), s] = sum_k Stat[k, (b,h)] * K2e2[k, s]
                pF = psum.tile([128, 64], f32, tag="pF")
                nc.tensor.matmul(
                    out=pF[:, :], lhsT=stat[:, :], rhs=s_K2e2,
                    start=True, stop=True,
                )

                # gate = sigmoid(pF + beta), then +1 on the idle ACT engine
                # (the last chunk folds the +1 into its DVE STT instead)
                gate = small.tile([128, 64], f32, tag="gate")
                nc.scalar.activation(gate[:, :], pF[:, :], AF.Sigmoid,
                                     bias=s_betacol)
                gate1 = None
                if ci != NCHUNK - 1:
                    gate1 = small.tile([128, 64], f32, tag="gate1")
                    nc.scalar.activation(gate1[:, :], gate[:, :], AF.Copy,
                                         bias=1.0)

                # out = (gate + 1) * x, gate broadcast over the 128 group
                # chans. Chunks 0..2 run it on GpSimd (the DVE is busy with
                # the next chunk's reduces, and 1-input DVE reduces leave the
                # shared DVE/GpSimd SBUF port free). The LAST chunk runs it
                # on the now-idle DVE via scalar_tensor_tensor, which is 2x
                # faster per element and shortens the tail. Sliced in 4
                # along c so each slice's DMA-out overlaps the next slice's
                # multiply.
                last = (ci == NCHUNK - 1)
                NSL = 8 if last else 4
                SL = FREE // NSL
                CSL = C16 // NSL
                for q in range(NSL):
                    f0 = q * SL
                    X3q = X[:, f0:f0 + SL].rearrange("p (c s) -> p c s", c=CSL)
                    if last:
                        g_bcq0 = gate[:, :].unsqueeze(1).broadcast_to(
                            [128, CSL, SPAT])
                        nc.vector.scalar_tensor_tensor(
                            out=X3q, in0=g_bcq0, scalar=1.0, in1=X3q,
                            op0=ALU.add, op1=ALU.mult,
                        )
                    else:
                        g_bcq = gate1[:, :].unsqueeze(1).broadcast_to(
                            [128, CSL, SPAT])
                        nc.gpsimd.tensor_mul(X3q, g_bcq, X3q)
                    nc.scalar.dma_start(out=od[ci, :, f0:f0 + SL],
                                        in_=X[:, f0:f0 + SL])

    _split_multi_waits(nc, mybir)
    return nc


def kernel(x, w1, b1, w2, b2, wv, bv, trace=False):
    global LAST_EXEC_NS
    from concourse.bass_utils import run_bass_kernel_spmd

    x = np.ascontiguousarray(np.asarray(x, np.float32))
    consts = _pack_params(w1, b1, w2, b2, wv, bv)

    nc = _build()

    in_maps = []
    for i in range(NCORES):
        shard = x[i * BPC:(i + 1) * BPC]  # [32, 2048, 8, 8]
        m = {"x": np.ascontiguousarray(shard.reshape(NCHUNK, 128, FREE))}
        m.update(consts)
        in_maps.append(m)

    res = run_bass_kernel_spmd(nc, in_maps, core_ids=list(range(NCORES)),
                               trace=trace)
    LAST_EXEC_NS = res.exec_time_ns

    out = np.empty((B, C, H, W), np.float32)
    for i, r in enumerate(res.results):
        out[i * BPC:(i + 1) * BPC] = r["out"].reshape(BPC, C, H, W)
    return out
